# revision 1
# baseline (speedup 1.0000x reference)
"""DeepSpeed-style fused residual+LayerNorm+MLP block on 8 trn2 NeuronCores.

Strategy: data-parallel over tokens (B*S = 16384 -> 2048 tokens/core).
Each core runs the full fused chain with replicated weights; no collectives.

Per-core device kernel (Bass/Tile), pipelined over 4 supertiles of 512 tokens:
  A1: h = x + r + bias; LayerNorm stats (bn_stats/bn_aggr); ln -> bf16
  A2: PE-transpose ln to feature-major lnT [H, tok]; psum->sbuf copies on
      ScalarE (Copy lives in every ACT table set -> no table reloads).
      For supertile s+1 the transposes ride in supertile s's GEMM1 tail so
      the PE HAM clock stays warm and the copies precede the tail GELUs.
  B:  interT[I,tok] = W1^T @ lnT (bf16 matmuls, fp32 PSUM);
      exact-erf GELU + per-I bias fused on ScalarE -> bf16
  C:  out[tok,H] = interT^T @ W2 (W2 streamed from HBM);
      epilogue adds h (+output_b) and DMAs out.

DMA queue assignment (avoids HWDGE FIFO head-of-line blocking):
  sync   : bias/identity broadcasts, x token loads, streamed W2 chunks
  gpsimd : output bias, GEMM1 bias, W1 (one-time) + out stores
           (stores wait on late semaphores; nothing queued behind them)

Host-side prep (cheap, numpy): fold attn_nw into W1 rows, fold
attn_nb@W1+inter_b into a single GEMM1 bias, cast weights to bf16,
provide the 128x128 bf16 identity for PE transposes.
"""

import numpy as np
import ml_dtypes

import concourse.bass as bass
import concourse.bacc as bacc
import concourse.mybir as mybir
import concourse.tile as tile
from concourse.bass_utils import run_bass_kernel_spmd

N_CORES = 8
B, S, H, I = 4, 4096, 1024, 4096
TOK = B * S              # 16384 tokens total
TPC = TOK // N_CORES     # 2048 tokens per core
P = 128
T_TILES = TPC // P       # 16 token tiles per core
ST = 4                   # token tiles per supertile
N_SUPER = T_TILES // ST  # 4 supertiles
ST_TOK = ST * P          # 512 tokens per supertile
KO1 = H // P             # 8 contraction subtiles for GEMM1
IC = I // P              # 32 I-chunks
IG = 8                   # W1 i-groups (independent SBUF tiles for early start)
ICG = IC // IG           # 4 I-chunks per group
HCW = 512                # output column chunk (1 PSUM bank of f32)
HC = H // HCW            # 2
EPS = 1e-5

_F32 = mybir.dt.float32
_BF16 = mybir.dt.bfloat16

TRACE = False
LAST_RESULT = None


def _build_nc():
    nc = bacc.Bacc()
    x = nc.dram_tensor("x", (TPC, H), _F32, kind="ExternalInput")
    r = nc.dram_tensor("r", (TPC, H), _F32, kind="ExternalInput")
    w1 = nc.dram_tensor("w1", (H, I), _BF16, kind="ExternalInput")
    b1 = nc.dram_tensor("b1", (I,), _F32, kind="ExternalInput")
    w2 = nc.dram_tensor("w2", (I, H), _BF16, kind="ExternalInput")
    ab = nc.dram_tensor("ab", (H,), _F32, kind="ExternalInput")
    ob = nc.dram_tensor("ob", (H,), _F32, kind="ExternalInput")
    eye = nc.dram_tensor("eye", (P, P), _BF16, kind="ExternalInput")
    out = nc.dram_tensor("out", (TPC, H), _F32, kind="ExternalOutput")

    with tile.TileContext(nc) as tc:
        with (
            tc.tile_pool(name="consts", bufs=1) as consts,
            tc.tile_pool(name="w1p", bufs=1) as w1p,
            tc.tile_pool(name="w2s", bufs=8) as w2s,
            tc.tile_pool(name="hsup", bufs=2) as hsup,
            tc.tile_pool(name="xin", bufs=3) as xin,
            tc.tile_pool(name="rin", bufs=3) as rin,
            tc.tile_pool(name="lnp", bufs=6) as lnp,
            tc.tile_pool(name="lntp", bufs=2) as lntp,
            tc.tile_pool(name="intp", bufs=1) as intp,
            tc.tile_pool(name="resp", bufs=4) as resp,
            tc.tile_pool(name="stat", bufs=8) as stat,
            tc.tile_pool(name="ps_tr", bufs=2, space="PSUM") as ps_tr,
            tc.tile_pool(name="ps_g1", bufs=2, space="PSUM") as ps_g1,
            tc.tile_pool(name="ps_g2", bufs=4, space="PSUM") as ps_g2,
        ):
            eps_t = consts.tile([P, 1], _F32)
            nc.vector.memset(eps_t, EPS)

            ab_full = consts.tile([P, H], _F32)
            ab_ap = ab[:]
            nc.gpsimd.dma_start(
                out=ab_full,
                in_=bass.AP(tensor=ab_ap.tensor, offset=ab_ap.offset,
                            ap=[[0, P]] + list(ab_ap.ap)),
            )
            ident = consts.tile([P, P], _BF16)
            nc.gpsimd.dma_start(out=ident, in_=eye[:, :])

            ob_full = consts.tile([P, H], _F32)
            b1_st = consts.tile([P, IC], _F32)
            nc.gpsimd.dma_start(out=b1_st, in_=b1[:].rearrange("(i p) -> p i", p=P))

            w1r = w1[:, :].rearrange("(ko p) i -> p ko i", p=P)
            w2r = w2[:, :].rearrange("(io p) h -> p io h", p=P)


            h_sups = [None] * N_SUPER
            lnTs = [None] * N_SUPER
            ln_ts = [None] * N_SUPER
            w1_ig = [None] * IG

            def emit_a1(s):
                """loads + residual adds + LN stats + normalized bf16 tiles"""
                h_sup = hsup.tile([P, ST, H], _F32, name=f"h_sup{s}", tag="h_sup")
                mv = stat.tile([P, ST, 2], _F32, name=f"mv_{s}", tag="mv")
                lns = []
                for t in range(ST):
                    g = s * ST + t
                    x_t = xin.tile([P, H], _F32, name=f"x_{g}", tag="x_t")
                    nc.sync.dma_start(out=x_t, in_=x[g * P:(g + 1) * P, :])
                    r_t = rin.tile([P, H], _F32, name=f"r_{g}", tag="r_t")
                    r_eng = nc.gpsimd if s == 0 else nc.sync
                    r_eng.dma_start(out=r_t, in_=r[g * P:(g + 1) * P, :])
                    h_sl = h_sup[:, t, :]
                    nc.vector.tensor_add(h_sl, x_t, r_t)
                    nc.vector.tensor_add(h_sl, h_sl, ab_full)
                    stats = stat.tile([P, 2, 6], _F32, name=f"st_{g}", tag="stats")
                    for q in range(2):
                        nc.vector.bn_stats(out=stats[:, q, :],
                                           in_=h_sl[:, q * 512:(q + 1) * 512])
                    nc.vector.bn_aggr(out=mv[:, t, :], in_=stats)
                    nc.scalar.activation(out=mv[:, t, 1:2], in_=mv[:, t, 1:2],
                                         func=mybir.ActivationFunctionType.Sqrt,
                                         bias=eps_t, scale=1.0)
                    nc.vector.reciprocal(out=mv[:, t, 1:2], in_=mv[:, t, 1:2])
                    ln_t = lnp.tile([P, H], _BF16, name=f"ln_{g}", tag="ln_t")
                    nc.vector.tensor_scalar(
                        out=ln_t, in0=h_sl,
                        scalar1=mv[:, t, 0:1], scalar2=mv[:, t, 1:2],
                        op0=mybir.AluOpType.subtract, op1=mybir.AluOpType.mult,
                    )
                    if s == N_SUPER - 1:
                        # fold output bias into h here; its epilogue is the
                        # kernel tail, so halving the tail DVE work matters
                        nc.vector.tensor_add(h_sl, h_sl, ob_full)
                    lns.append(ln_t)
                h_sups[s] = h_sup
                ln_ts[s] = lns
                lnTs[s] = lntp.tile([P, KO1, ST_TOK], _BF16, name=f"lnT{s}",
                                    tag="lnT")

            def emit_a2_one(s, idx):
                """one PE transpose + ACT psum->sbuf copy (idx in [0, ST*KO1))"""
                t, k = divmod(idx, KO1)
                trp = ps_tr.tile([P, P], _BF16, name=f"tr_{s}_{idx}", tag="trp")
                nc.tensor.transpose(trp, ln_ts[s][t][:, k * P:(k + 1) * P], ident)
                nc.scalar.copy(out=lnTs[s][:, k, t * P:(t + 1) * P], in_=trp)

            def emit_b(s, interleave_a2):
                """GEMM1 + bias + exact GELU -> interT; the next supertile's
                transposes ride along in the last i-chunks so their ACT
                copies precede the tail GELUs in queue order."""
                interT = intp.tile([P, IC, ST_TOK], _BF16, name=f"interT{s}",
                                   tag="interT")
                lnT = lnTs[s]
                a2_idx = 0
                for i in range(IC):
                    pg1 = ps_g1.tile([P, ST_TOK], _F32, name=f"pg1_{s}_{i}",
                                     tag="pg1")
                    for k in range(KO1):
                        nc.tensor.matmul(pg1,
                                         w1_ig[i // ICG][:, k, (i % ICG) * P:
                                                         (i % ICG + 1) * P],
                                         lnT[:, k, :],
                                         start=(k == 0), stop=(k == KO1 - 1))
                    if interleave_a2 is not None and i >= IC - 2 * IG:
                        for _ in range(ST * KO1 // (2 * IG)):
                            if a2_idx < ST * KO1:
                                emit_a2_one(interleave_a2, a2_idx)
                                a2_idx += 1
                    nc.scalar.activation(out=interT[:, i, :], in_=pg1,
                                         func=mybir.ActivationFunctionType.Gelu,
                                         bias=b1_st[:, i:i + 1], scale=1.0)
                return interT

            def emit_c(s, interT):
                """GEMM2 (W2 streamed) + epilogue"""
                for hc in range(HC):
                    pg2s = [ps_g2.tile([P, HCW], _F32, name=f"pg2_{s}_{hc}_{tq}",
                                       tag="pg2")
                            for tq in range(ST)]
                    for i in range(IC):
                        w2c = w2s.tile([P, HCW], _BF16, name=f"w2c_{s}_{hc}_{i}",
                                       tag="w2c")
                        nc.sync.dma_start(out=w2c,
                                          in_=w2r[:, i, hc * HCW:(hc + 1) * HCW])
                        for tq in range(ST):
                            nc.tensor.matmul(pg2s[tq],
                                             interT[:, i, tq * P:(tq + 1) * P],
                                             w2c,
                                             start=(i == 0), stop=(i == IC - 1))
                    for tq in range(ST):
                        g = s * ST + tq
                        res_h = resp.tile([P, HCW], _F32, name=f"res_{s}_{hc}_{tq}",
                                          tag="res_h")
                        nc.vector.tensor_add(res_h, pg2s[tq],
                                             h_sups[s][:, tq, hc * HCW:(hc + 1) * HCW])
                        if s != N_SUPER - 1:
                            nc.vector.tensor_add(res_h, res_h,
                                                 ob_full[:, hc * HCW:(hc + 1) * HCW])
                        # final supertile: split stores across both queues --
                        # sync is idle once the last W2 chunks are in, and the
                        # stores are the kernel tail
                        st_eng = (nc.sync if (s == N_SUPER - 1 and hc == 1)
                                  else nc.gpsimd)
                        st_eng.dma_start(
                            out=out[g * P:(g + 1) * P, hc * HCW:(hc + 1) * HCW],
                            in_=res_h)

            # ---- emission schedule ----
            emit_a1(0)                      # token loads queue first after consts
            for ig in range(IG):            # W1 on the gpsimd queue, in 8 groups
                w1t = w1p.tile([P, KO1, ICG * P], _BF16, name=f"w1_{ig}",
                               tag=f"w1_{ig}")
                kh = KO1 // 2
                for q in range(2):
                    nc.gpsimd.dma_start(
                        out=w1t[:, q * kh:(q + 1) * kh, :],
                        in_=w1r[:, q * kh:(q + 1) * kh,
                                ig * ICG * P:(ig + 1) * ICG * P])
                w1_ig[ig] = w1t
            ob_ap = ob[:]
            nc.gpsimd.dma_start(
                out=ob_full,
                in_=bass.AP(tensor=ob_ap.tensor, offset=ob_ap.offset,
                            ap=[[0, P]] + list(ob_ap.ap)),
            )
            for idx in range(ST * KO1):     # supertile 0 transposes up front
                emit_a2_one(0, idx)
            for s in range(N_SUPER):
                if s + 1 < N_SUPER:
                    emit_a1(s + 1)
                interT = emit_b(s, s + 1 if s + 1 < N_SUPER else None)
                emit_c(s, interT)

    nc.finalize()
    return nc


def kernel(input, residual, bias, attn_nw, attn_nb, inter_w, inter_b,
           output_w, output_b):
    global LAST_RESULT
    input = np.asarray(input, dtype=np.float32)
    residual = np.asarray(residual, dtype=np.float32)
    bias = np.asarray(bias, dtype=np.float32)
    attn_nw = np.asarray(attn_nw, dtype=np.float32)
    attn_nb = np.asarray(attn_nb, dtype=np.float32)
    inter_w = np.asarray(inter_w, dtype=np.float32)
    inter_b = np.asarray(inter_b, dtype=np.float32)
    output_w = np.asarray(output_w, dtype=np.float32)
    output_b = np.asarray(output_b, dtype=np.float32)

    x = np.ascontiguousarray(input.reshape(TOK, H))
    r = np.ascontiguousarray(residual.reshape(TOK, H))
    # fold LN affine params into GEMM1 weight/bias (exact algebra):
    #   (std*nw + nb) @ W1 + b1 == std @ (nw[:,None]*W1) + (nb @ W1 + b1)
    w1p = np.ascontiguousarray((attn_nw[:, None] * inter_w)).astype(ml_dtypes.bfloat16)
    b1p = (attn_nb @ inter_w + inter_b).astype(np.float32)
    w2p = np.ascontiguousarray(output_w).astype(ml_dtypes.bfloat16)
    eye = np.eye(P, dtype=ml_dtypes.bfloat16)

    nc = _build_nc()
    in_maps = []
    for c in range(N_CORES):
        in_maps.append({
            "x": np.ascontiguousarray(x[c * TPC:(c + 1) * TPC]),
            "r": np.ascontiguousarray(r[c * TPC:(c + 1) * TPC]),
            "w1": w1p, "b1": b1p, "w2": w2p,
            "ab": bias, "ob": output_b, "eye": eye,
        })
    res = run_bass_kernel_spmd(nc, in_maps, core_ids=list(range(N_CORES)),
                               trace=TRACE)
    LAST_RESULT = res
    out = np.concatenate([res.results[c]["out"] for c in range(N_CORES)], axis=0)
    return np.ascontiguousarray(out.reshape(B, S, H)).astype(np.float32)



# revision 7
# speedup vs baseline: 1.8010x; 1.8010x over previous
"""DeepSpeed-style fused residual+LayerNorm+MLP block on 8 trn2 NeuronCores.

Strategy: data-parallel over tokens (B*S = 16384 -> 2048 tokens/core).
Each core runs the full fused chain with replicated weights; no collectives.

fp8 (e4m3) tensor path: both GEMMs run in MatmulPerfMode.DoubleRow (two
128-deep k-tiles per pass -> 2x bf16 throughput). Weights are pre-scaled
by S_W=1024 host-side so they sit in fp8's normal range; the scale is
undone in the GELU (scale=1/S_W) and in the GEMM2 epilogue copy.
Numerics: the 2e-2 rel-err budget absorbs the fp8 quantization error
(~1.8e-2 simulated; the fp32 residual h dominates the output norm).

CRITICAL HW constraint: mixing normal-mode PE instructions (e.g.
is_transpose or bf16 matmuls) into a stream of DoubleRow matmuls
corrupts the adjacent DoubleRow results nondeterministically (LDWEIGHTS
pull-ahead across perf modes; a mid-chain mix even hard-wedges the
core). The ENTIRE PE stream here is therefore DoubleRow fp8 matmuls --
the ln transpose is itself expressed as one:
  out[h,t] = ln_blk.T @ I  via  lhsT=[ln_k, ln_k+1], rhs=[I, 0]
(the zeroed second moving slot annihilates the junk second product; the
one past-the-end pad block is memset to avoid NaN*0).

Per-core device kernel (Bass/Tile), pipelined over 4 supertiles of 512
tokens:
  A1: h = x + r + bias; LayerNorm stats (bn_stats/bn_aggr); ln -> fp8
      (DVE writes fp8 directly); output bias folded into h after ln
  A2: DoubleRow transpose matmuls -> f32 PSUM; ACT copies convert to
      the fp8 feature-major lnT [H, tok]. Supertile s+1's transposes
      ride in supertile s's GEMM1 tail so the PE stays warm and the
      copies precede the tail GELUs.
  B:  interT[I,tok] = W1^T @ lnT (fp8 DoubleRow, fp32 PSUM); exact-erf
      GELU with scale=1/S_W + per-I bias fused on ScalarE -> fp8
  C:  out[tok,H] = interT^T @ W2 (W2 resident in SBUF, fp8 DoubleRow);
      ACT copy applies 1/S_W, DVE adds h (+output_b already folded),
      DMA out.

DMA queue assignment:
  sync   : x/r token loads, final-tile stores
  gpsimd : consts, W1 + W2 (one-time), r supertile 0, out stores

Host-side prep (cheap, numpy): fold attn_nw into W1 rows, fold
attn_nb@W1+inter_b into a single GEMM1 bias, scale weights by S_W and
cast to fp8 e4m3, provide the [identity; zeros] fp8 pair tile.
"""

import numpy as np
import ml_dtypes

import concourse.bass as bass
import concourse.bacc as bacc
import concourse.mybir as mybir
import concourse.tile as tile
from concourse.bass_utils import run_bass_kernel_spmd

N_CORES = 8
B, S, H, I = 4, 4096, 1024, 4096
TOK = B * S              # 16384 tokens total
TPC = TOK // N_CORES     # 2048 tokens per core
P = 128
T_TILES = TPC // P       # 16 token tiles per core
ST = 4                   # token tiles per supertile
N_SUPER = T_TILES // ST  # 4 supertiles
ST_TOK = ST * P          # 512 tokens per supertile
KO1 = H // P             # 8 contraction subtiles for GEMM1
KP1 = KO1 // 2           # 4 DoubleRow pairs for GEMM1
IC = I // P              # 32 I-chunks
IP2 = IC // 2            # 16 DoubleRow pairs for GEMM2
IG = 8                   # W1 i-groups (independent SBUF tiles for early start)
ICG = IC // IG           # 4 I-chunks per group
HCW = 512                # output column chunk (1 PSUM bank of f32)
HC = H // HCW            # 2
EPS = 1e-5
S_W = 1024.0             # host-side fp8 weight scale

_F32 = mybir.dt.float32
_F8 = mybir.dt.float8e4
_DR = mybir.MatmulPerfMode.DoubleRow

TRACE = False
LAST_RESULT = None


def _build_nc():
    nc = bacc.Bacc()
    x = nc.dram_tensor("x", (TPC, H), _F32, kind="ExternalInput")
    r = nc.dram_tensor("r", (TPC, H), _F32, kind="ExternalInput")
    w1 = nc.dram_tensor("w1", (H, I), _F8, kind="ExternalInput")
    b1 = nc.dram_tensor("b1", (I,), _F32, kind="ExternalInput")
    w2 = nc.dram_tensor("w2", (I, H), _F8, kind="ExternalInput")
    ab = nc.dram_tensor("ab", (H,), _F32, kind="ExternalInput")
    ob = nc.dram_tensor("ob", (H,), _F32, kind="ExternalInput")
    eye = nc.dram_tensor("eye", (P, 2, P), _F8, kind="ExternalInput")
    out = nc.dram_tensor("out", (TPC, H), _F32, kind="ExternalOutput")

    with tile.TileContext(nc) as tc:
        with (
            tc.tile_pool(name="consts", bufs=1) as consts,
            tc.tile_pool(name="w1p", bufs=1) as w1p,
            tc.tile_pool(name="w2p", bufs=1) as w2p,
            tc.tile_pool(name="hsup", bufs=2) as hsup,
            tc.tile_pool(name="xin", bufs=3) as xin,
            tc.tile_pool(name="rin", bufs=3) as rin,
            tc.tile_pool(name="lnp", bufs=6) as lnp,
            tc.tile_pool(name="lntp", bufs=2) as lntp,
            tc.tile_pool(name="intp", bufs=1) as intp,
            tc.tile_pool(name="resp", bufs=4) as resp,
            tc.tile_pool(name="stat", bufs=8) as stat,
            tc.tile_pool(name="ps_tr", bufs=2, space="PSUM") as ps_tr,
            tc.tile_pool(name="ps_g1", bufs=2, space="PSUM") as ps_g1,
            tc.tile_pool(name="ps_g2", bufs=4, space="PSUM") as ps_g2,
        ):
            eps_t = consts.tile([P, 1], _F32)
            nc.vector.memset(eps_t, EPS)

            ab_full = consts.tile([P, H], _F32)
            ab_ap = ab[:]
            nc.gpsimd.dma_start(
                out=ab_full,
                in_=bass.AP(tensor=ab_ap.tensor, offset=ab_ap.offset,
                            ap=[[0, P]] + list(ab_ap.ap)),
            )
            ident2 = consts.tile([P, 2, P], _F8)   # [identity, zeros]
            nc.gpsimd.dma_start(out=ident2, in_=eye[:, :, :])

            ob_full = consts.tile([P, H], _F32)
            # the ob DMA must be emitted BEFORE emit_a1(0) reads ob_full:
            # a read emitted before the write gets no dependency edge
            ob_ap = ob[:]
            nc.gpsimd.dma_start(
                out=ob_full,
                in_=bass.AP(tensor=ob_ap.tensor, offset=ob_ap.offset,
                            ap=[[0, P]] + list(ob_ap.ap)),
            )
            b1_st = consts.tile([P, IC], _F32)
            nc.gpsimd.dma_start(out=b1_st, in_=b1[:].rearrange("(i p) -> p i", p=P))

            w1r = w1[:, :].rearrange("(ko p) i -> p ko i", p=P)
            w2r = w2[:, :].rearrange("(io p) h -> p io h", p=P)

            h_sups = [None] * N_SUPER
            lnTs = [None] * N_SUPER
            ln_ts = [None] * N_SUPER
            w1_ig = [None] * IG
            w2_hc = [None] * HC

            def emit_a1(s):
                """loads + residual adds + LN stats + normalized fp8 tiles"""
                h_sup = hsup.tile([P, ST, H], _F32, name=f"h_sup{s}", tag="h_sup")
                mv = stat.tile([P, ST, 2], _F32, name=f"mv_{s}", tag="mv")
                lns = []
                x_ts, r_ts = [], []
                for t in range(ST):
                    g = s * ST + t
                    x_t = xin.tile([P, H], _F32, name=f"x_{g}", tag="x_t")
                    nc.sync.dma_start(out=x_t, in_=x[g * P:(g + 1) * P, :])
                    r_t = rin.tile([P, H], _F32, name=f"r_{g}", tag="r_t")
                    r_eng = nc.gpsimd if s == 0 else nc.sync
                    r_eng.dma_start(out=r_t, in_=r[g * P:(g + 1) * P, :])
                    x_ts.append(x_t)
                    r_ts.append(r_t)
                for t in range(ST):
                    g = s * ST + t
                    h_sl = h_sup[:, t, :]
                    nc.vector.tensor_add(h_sl, x_ts[t], r_ts[t])
                    nc.vector.tensor_add(h_sl, h_sl, ab_full)
                    stats = stat.tile([P, 2, 6], _F32, name=f"st_{g}", tag="stats")
                    for q in range(2):
                        nc.vector.bn_stats(out=stats[:, q, :],
                                           in_=h_sl[:, q * 512:(q + 1) * 512])
                    nc.vector.bn_aggr(out=mv[:, t, :], in_=stats)
                    nc.scalar.activation(out=mv[:, t, 1:2], in_=mv[:, t, 1:2],
                                         func=mybir.ActivationFunctionType.Sqrt,
                                         bias=eps_t, scale=1.0)
                    nc.vector.reciprocal(out=mv[:, t, 1:2], in_=mv[:, t, 1:2])
                    # ln tile has one extra 128-block: the k=7 transpose pair
                    # reads past the last block; memset keeps NaNs out of the
                    # zero-multiplied slot
                    ln_t = lnp.tile([P, KO1 + 1, P], _F8, name=f"ln_{g}",
                                    tag="ln_t")
                    nc.vector.tensor_scalar(
                        out=ln_t[:, :KO1, :], in0=h_sl,
                        scalar1=mv[:, t, 0:1], scalar2=mv[:, t, 1:2],
                        op0=mybir.AluOpType.subtract, op1=mybir.AluOpType.mult,
                    )
                    nc.vector.memset(ln_t[:, KO1, :], 0.0)
                    # fold output bias into h after ln is taken; the GEMM2
                    # epilogue then only needs one DVE add per tile
                    nc.vector.tensor_add(h_sl, h_sl, ob_full)
                    lns.append(ln_t)
                h_sups[s] = h_sup
                ln_ts[s] = lns
                lnTs[s] = lntp.tile([P, KO1, ST_TOK], _F8, name=f"lnT{s}",
                                    tag="lnT")

            def emit_a2_one(s, idx):
                """one DoubleRow transpose matmul + ACT psum->sbuf fp8 copy:
                lhsT=[ln_k, ln_k+1], rhs=[I, 0] -> psum = ln_k.T (f32)"""
                t, k = divmod(idx, KO1)
                trp = ps_tr.tile([P, P], _F32, name=f"tr_{s}_{idx}", tag="trp")
                nc.tensor.matmul(trp, ln_ts[s][t][:, k:k + 2, :],
                                 ident2[:, 0:2, :],
                                 start=True, stop=True, perf_mode=_DR)
                nc.scalar.copy(out=lnTs[s][:, k, t * P:(t + 1) * P], in_=trp)

            def emit_b(s, interleave_a2):
                """GEMM1 (fp8 DoubleRow) + scaled bias + exact GELU -> interT;
                the next supertile's transposes ride along in the last
                i-chunks so their ACT copies precede the tail GELUs."""
                interT = intp.tile([P, IC, ST_TOK], _F8, name=f"interT{s}",
                                   tag="interT")
                lnT = lnTs[s]
                a2_idx = 0
                for i in range(IC):
                    pg1 = ps_g1.tile([P, ST_TOK], _F32, name=f"pg1_{s}_{i}",
                                     tag="pg1")
                    wg = w1_ig[i // ICG]
                    iw = (i % ICG) * P
                    for kp in range(KP1):
                        nc.tensor.matmul(pg1,
                                         wg[:, 2 * kp:2 * kp + 2, iw:iw + P],
                                         lnT[:, 2 * kp:2 * kp + 2, :],
                                         start=(kp == 0), stop=(kp == KP1 - 1),
                                         perf_mode=_DR)
                    if interleave_a2 is not None and i >= IC - 2 * IG:
                        for _ in range(ST * KO1 // (2 * IG)):
                            if a2_idx < ST * KO1:
                                emit_a2_one(interleave_a2, a2_idx)
                                a2_idx += 1
                    nc.scalar.activation(out=interT[:, i, :], in_=pg1,
                                         func=mybir.ActivationFunctionType.Gelu,
                                         bias=b1_st[:, i:i + 1], scale=1.0 / S_W)
                return interT

            def emit_c(s, interT):
                """GEMM2 (fp8 DoubleRow, W2 resident) + epilogue"""
                for hc in range(HC):
                    w2t = w2_hc[hc]
                    for tq in range(ST):
                        pg2 = ps_g2.tile([P, HCW], _F32, name=f"pg2_{s}_{hc}_{tq}",
                                         tag="pg2")
                        for ip in range(IP2):
                            nc.tensor.matmul(pg2,
                                             interT[:, 2 * ip:2 * ip + 2,
                                                    tq * P:(tq + 1) * P],
                                             w2t[:, 2 * ip:2 * ip + 2, :],
                                             start=(ip == 0), stop=(ip == IP2 - 1),
                                             perf_mode=_DR)
                        g = s * ST + tq
                        res_h = resp.tile([P, HCW], _F32, name=f"res_{s}_{hc}_{tq}",
                                          tag="res_h")
                        nc.scalar.activation(out=res_h, in_=pg2,
                                             func=mybir.ActivationFunctionType.Copy,
                                             scale=1.0 / S_W)
                        nc.vector.tensor_add(res_h, res_h,
                                             h_sups[s][:, tq, hc * HCW:(hc + 1) * HCW])
                        # final supertile: split stores across both queues --
                        # sync is idle once the last x/r tiles are in, and the
                        # stores are the kernel tail
                        st_eng = (nc.sync if (s == N_SUPER - 1 and hc == 1)
                                  else nc.gpsimd)
                        st_eng.dma_start(
                            out=out[g * P:(g + 1) * P, hc * HCW:(hc + 1) * HCW],
                            in_=res_h)

            # ---- emission schedule ----
            emit_a1(0)                      # token loads queue first after consts
            for ig in range(IG):            # W1 on the gpsimd queue, in 8 groups
                w1t = w1p.tile([P, KO1, ICG * P], _F8, name=f"w1_{ig}",
                               tag=f"w1_{ig}")
                kh = KO1 // 2
                for q in range(2):
                    nc.gpsimd.dma_start(
                        out=w1t[:, q * kh:(q + 1) * kh, :],
                        in_=w1r[:, q * kh:(q + 1) * kh,
                                ig * ICG * P:(ig + 1) * ICG * P])
                w1_ig[ig] = w1t
            for hc in range(HC):            # W2 resident in SBUF, H-half chunks
                w2t = w2p.tile([P, IC, HCW], _F8, name=f"w2_{hc}", tag=f"w2_{hc}")
                nc.gpsimd.dma_start(out=w2t,
                                    in_=w2r[:, :, hc * HCW:(hc + 1) * HCW])
                w2_hc[hc] = w2t
            for idx in range(ST * KO1):     # supertile 0 transposes up front
                emit_a2_one(0, idx)
            for s in range(N_SUPER):
                if s + 1 < N_SUPER:
                    emit_a1(s + 1)
                interT = emit_b(s, s + 1 if s + 1 < N_SUPER else None)
                emit_c(s, interT)

    nc.finalize()
    return nc


def kernel(input, residual, bias, attn_nw, attn_nb, inter_w, inter_b,
           output_w, output_b):
    global LAST_RESULT
    input = np.asarray(input, dtype=np.float32)
    residual = np.asarray(residual, dtype=np.float32)
    bias = np.asarray(bias, dtype=np.float32)
    attn_nw = np.asarray(attn_nw, dtype=np.float32)
    attn_nb = np.asarray(attn_nb, dtype=np.float32)
    inter_w = np.asarray(inter_w, dtype=np.float32)
    inter_b = np.asarray(inter_b, dtype=np.float32)
    output_w = np.asarray(output_w, dtype=np.float32)
    output_b = np.asarray(output_b, dtype=np.float32)

    x = np.ascontiguousarray(input.reshape(TOK, H))
    r = np.ascontiguousarray(residual.reshape(TOK, H))
    # fold LN affine params into GEMM1 weight/bias (exact algebra):
    #   (std*nw + nb) @ W1 + b1 == std @ (nw[:,None]*W1) + (nb @ W1 + b1)
    # weights scaled by S_W so fp8 e4m3 sees normal-range values; the
    # device kernel divides the PSUM results back down
    w1p = np.ascontiguousarray(attn_nw[:, None] * inter_w * S_W).astype(
        ml_dtypes.float8_e4m3)
    b1p = (attn_nb @ inter_w + inter_b).astype(np.float32)
    w2p = np.ascontiguousarray(output_w * S_W).astype(ml_dtypes.float8_e4m3)
    eye2 = np.zeros((P, 2, P), dtype=ml_dtypes.float8_e4m3)
    eye2[:, 0, :] = np.eye(P, dtype=np.float32).astype(ml_dtypes.float8_e4m3)

    nc = _build_nc()
    in_maps = []
    for c in range(N_CORES):
        in_maps.append({
            "x": np.ascontiguousarray(x[c * TPC:(c + 1) * TPC]),
            "r": np.ascontiguousarray(r[c * TPC:(c + 1) * TPC]),
            "w1": w1p, "b1": b1p, "w2": w2p,
            "ab": bias, "ob": output_b, "eye": eye2,
        })
    res = run_bass_kernel_spmd(nc, in_maps, core_ids=list(range(N_CORES)),
                               trace=TRACE)
    LAST_RESULT = res
    out = np.concatenate([res.results[c]["out"] for c in range(N_CORES)], axis=0)
    return np.ascontiguousarray(out.reshape(B, S, H)).astype(np.float32)
